# revision 30
# baseline (speedup 1.0000x reference)
"""Trainium2 Bass kernel: batched multi-head scaled-dot-product attention.

Problem shapes: Q/K/V [4, 16, 2048, 64] f32, mask [4, 1, 2048, 2048] bool.
out = softmax(Q K^T / 8 + mask) V.

Sharding: 8 cores; core c handles batch b = c//2, heads (c%2)*8 .. +8.
Each (b, h) is independent; the mask is shared across the 8 heads on a core.

Per-core kernel (per head h, per query-half qh of 1024):
  S^T[k, q]  = K Q^T         (PE; d=64 on partitions, 16 key-blocks of 128)
  P^T        = exp(S^T / 8)  (ACT, from PSUM)
  PM^T       = P^T * mask^T  (0/1 multiply; DVE + GPSIMD split; exact)
  acc[d', q] = sum_k V'[k, d'] PM^T[k, q]  (PE, PSUM accumulate over k-blocks;
               V' has a ones column so acc[64, q] = softmax denominator)
  out[q, d]  = transpose(acc) * 1/acc[64]  (PE transpose + DVE recip/mul)

QK^T and PV run as float32r (1 PE cycle/row vs 4 for float32).
"""

import numpy as np
import ml_dtypes

import bass_rust
import concourse.bass as bass
import concourse.mybir as mybir
import concourse.tile as tile
from concourse.bass_utils import run_bass_kernel_spmd
from concourse.masks import make_identity

B, H, S, D = 4, 16, 2048, 64
N_CORES = 8
HPC = H // (N_CORES // B)  # heads per core = 8
KB = S // 128  # 16 key blocks
QH = 2  # query halves
QHS = S // QH  # 1024
F32 = mybir.dt.float32
F32R = mybir.dt.float32r
BF16 = mybir.dt.bfloat16

# Kernel configuration (tuned via TimelineSim + hardware runs).
CONFIG = {
    "p_bf16": True,  # P (post-exp) and V in bf16: faster DVE mask, ~10x err
    "dve_cols": 1024,  # mask columns (of 1024) on DVE; GPSIMD op was slow on HW
    "p_bufs": 10,
    "s_bufs": 2,
    "pv_skew": 2,  # deferred-PE-queue depth (PE is in-order)
    "reps": 1,  # repeat the whole body (timing experiments only)
    "norm_on_act": False,  # ScalarE copies measured neutral; keep DVE path
    "mask_on_pe": False,  # PE-side additive mask measured +51us/body on HW
    "mask_swdge": False,  # SWDGE cast-DMA: Q7 does the u8->bf16 cast in
    # software, ~12ms one-time on HW (sim doesn't model it). False = HWDGE
    # u8 load + on-chip DVE cast.
    "qk_bf16": True,  # Q/K in bf16: PE QK^T at true 1 cyc/row on HW
    "mask_off": False,  # timing ablation: skip mask load + cast chain
    "mask_bits": True,  # ship mask bit-packed (S/8 bytes/row), unpack on DVE
    "out_bf16": True,  # store out bf16: halves D2H, ~2e-3 extra rounding
}


def _patched_drain_and_barrier(self, tick_clock, wait_clock):
    """This neuronxcc's CoreV3 codegen allows only 1 sync-wait per TPB_CTRL
    instruction; Tile's end-of-kernel drain can carry many. Split them."""
    drain_inst = self.nc.sync.drain()
    wait_clock.add_sem_waits(
        drain_inst.ins, tile.ScopedClock({None: tick_clock.global_clock})
    )
    mi = drain_inst.ins
    si = mi.sync_info
    waits = list(si.on_wait) if si is not None else []
    if len(waits) > 1:
        si.on_wait = waits[:1]
        mi.sync_info = si
        for i in range(1, len(waits)):
            extra = self.nc.sync.drain()
            extra.ins.sync_info = bass_rust.SyncInfo(
                on_wait=waits[i : i + 1], on_update=[]
            )
    self.nc.all_engine_barrier()
    popped = self.nc._tile_sem_poison_stack.pop()
    assert popped is self._sem_poison
    self.nc.clear_and_free_semaphores(list(self.sems.allocated().values()))
    self.nc.all_engine_barrier()


tile.TileContext._drain_and_barrier = _patched_drain_and_barrier

_ORIG_COMMIT = tile.TileContext._commit_instruction


def _commit_split_waits(self, inst, lazy_reg_writes=True):
    """Hoist all-but-one sem wait of an instruction onto single-wait NoOp
    carriers on the same engine (same 1-wait codegen limit as above).
    DMACopy lowers to SP pseudo-DMA instructions dispatched in program order
    by the SP sequencer, so carriers gate those too."""
    si = getattr(inst, "sync_info", None)
    if (
        si is not None
        and len(si.on_wait) > 1
        and inst.engine != mybir.EngineType.Unassigned
    ):
        waits = list(si.on_wait)
        for w in waits[:-1]:
            nop = mybir.InstNoOp(name=self.nc.get_next_instruction_name())
            nop.engine = inst.engine
            nop.sync_info = bass_rust.SyncInfo(on_wait=[w], on_update=[])
            self._add_instruction(nop)
        si.on_wait = waits[-1:]
        inst.sync_info = si
    return _ORIG_COMMIT(self, inst, lazy_reg_writes)


tile.TileContext._commit_instruction = _commit_split_waits

_NC_CACHE = {}


def build_nc(**overrides):
    cfg = dict(CONFIG)
    cfg.update(overrides)
    key = tuple(sorted(cfg.items()))
    if key in _NC_CACHE:
        return _NC_CACHE[key]
    p_dt = BF16 if cfg["p_bf16"] else F32R
    v_dt = BF16 if cfg["p_bf16"] else F32R
    qk_dt = BF16 if cfg["qk_bf16"] else F32R
    dve_cols = cfg["dve_cols"]

    nc = bass.Bass("TRN2", target_bir_lowering=False, debug=False, num_devices=N_CORES)
    qT = nc.dram_tensor("qT", [HPC, D, S], qk_dt, kind="ExternalInput")
    kT = nc.dram_tensor("kT", [HPC, D, S], qk_dt, kind="ExternalInput")
    # v pre-permuted on host to [h, p, kb, d] (p = key % 128, kb = key // 128)
    # so the SBUF load is 128 contiguous per-partition descriptors, not 2048
    # tiny strided ones. out likewise stored as [h, qh, p, j, d] (q = j*128+p)
    # and unpermuted on host.
    v = nc.dram_tensor("v", [HPC, 128, KB, D + 1], v_dt, kind="ExternalInput")
    if cfg["mask_bits"]:
        # bit-packed + host-permuted: maskT[p, kb, i] byte holds keys
        # kb*128+p, queries i*8..i*8+7 (little-endian bit order)
        maskT = nc.dram_tensor(
            "maskT", [128, KB, S // 8], mybir.dt.uint8, kind="ExternalInput"
        )
    else:
        maskT = nc.dram_tensor("maskT", [S, S], mybir.dt.uint8, kind="ExternalInput")
    out_dt = BF16 if cfg["out_bf16"] else F32
    out = nc.dram_tensor(
        "out", [HPC, QH, 128, QHS // 128, D], out_dt, kind="ExternalOutput"
    )

    with tile.TileContext(nc) as tc:
        with (
            tc.tile_pool(name="consts", bufs=1) as consts,
            tc.tile_pool(name="qk", bufs=2) as qk_pool,
            tc.tile_pool(name="vp", bufs=2) as v_pool,
            tc.tile_pool(name="pp", bufs=cfg["p_bufs"]) as p_pool,
            tc.tile_pool(name="pm", bufs=cfg["p_bufs"]) as pm_pool,
            tc.tile_pool(name="op", bufs=2) as o_pool,
            tc.tile_pool(name="small", bufs=2) as small,
            tc.tile_pool(name="res", bufs=2) as res_pool,
            tc.tile_pool(name="ps_s", bufs=cfg["s_bufs"], space="PSUM") as ps_s,
            tc.tile_pool(name="ps_acc", bufs=1, space="PSUM") as ps_acc,
            tc.tile_pool(name="ps_tr", bufs=1, space="PSUM") as ps_tr,
        ):
            identity = consts.tile([128, 128], F32)
            make_identity(nc, identity)
            mask_on_pe = cfg["mask_on_pe"]
            if mask_on_pe:
                identity_bf = consts.tile([128, 128], BF16)
                make_identity(nc, identity_bf)
            mask_sb = consts.tile([128, KB, S], BF16)
            if cfg["mask_bits"]:
                mT = maskT
            else:
                mT = maskT.rearrange("(n p) q -> p n q", p=128)

            def load_mask():
                if cfg["mask_bits"]:
                    mu8p = consts.tile(
                        [128, KB, S // 8], mybir.dt.uint8, name="mask_u8p"
                    )
                    nc.sync.dma_start(out=mu8p, in_=mT[:, :, :])
                    mu8 = consts.tile([128, KB, S], mybir.dt.uint8, name="mask_u8")
                    src = mu8p.rearrange("p n i -> p (n i)")
                    dst = mu8.rearrange("p n (i j) -> p (n i) j", j=8)
                    for j in range(8):
                        # bitVec ops can't cast, so extract bits u8->u8
                        nc.vector.tensor_scalar(
                            dst[:, :, j],
                            src,
                            j,
                            1,
                            mybir.AluOpType.logical_shift_right,
                            mybir.AluOpType.bitwise_and,
                        )
                    for c4 in range(4):
                        kbs = slice(c4 * 4, (c4 + 1) * 4)
                        nc.vector.tensor_copy(mask_sb[:, kbs, :], mu8[:, kbs, :])
                else:
                    mu8 = consts.tile([128, KB, S], mybir.dt.uint8, name="mask_u8")
                    for c4 in range(4):
                        kbs = slice(c4 * 4, (c4 + 1) * 4)
                        nc.sync.dma_start(out=mu8[:, kbs, :], in_=mT[:, kbs, :])
                        nc.vector.tensor_copy(mask_sb[:, kbs, :], mu8[:, kbs, :])

            skew = cfg["pv_skew"]
            # Deferred PE work (PV accumulates + normalizes), drained one item
            # per QK slot including across (h, qh) boundaries — keeps the
            # in-order PE from stalling on a just-produced dependency.
            from collections import deque

            pe_queue = deque()

            def drain_pe(target):
                while len(pe_queue) > target:
                    pe_queue.popleft()()

            def make_norm(h, qh, q0, acc, ntag):
                def norm():
                    # normalize: transpose acc to [q, d], divide by ones-col
                    o_sb = o_pool.tile([D + 1, QHS], F32, tag="o", name=f"o_{ntag}_{qh}")
                    cp = nc.scalar if cfg["norm_on_act"] else nc.vector
                    cp.copy(o_sb, acc) if cfg["norm_on_act"] else cp.tensor_copy(o_sb, acc)
                    tr = ps_tr.tile([128, 8, 128], F32, tag="tr", name=f"tr_{ntag}_{qh}")
                    for j in range(8):
                        nc.tensor.transpose(
                            tr[:, j, 0 : D + 1],
                            o_sb[:, j * 128 : (j + 1) * 128],
                            identity[0 : D + 1, 0 : D + 1],
                        )
                    den_sb = small.tile([128, 8], F32, tag="den", name=f"den_{ntag}_{qh}")
                    if cfg["norm_on_act"]:
                        nc.scalar.copy(den_sb, tr[:, :, D])
                    else:
                        nc.vector.tensor_copy(den_sb, tr[:, :, D])
                    rec_sb = small.tile([128, 8], F32, tag="rec", name=f"rec_{ntag}_{qh}")
                    nc.vector.reciprocal(rec_sb, den_sb)
                    res_sb = res_pool.tile(
                        [128, 8, D], out_dt, tag="res", name=f"res_{ntag}_{qh}"
                    )
                    for j in range(8):
                        nc.vector.tensor_scalar_mul(
                            res_sb[:, j, :], tr[:, j, 0:D], rec_sb[:, j : j + 1]
                        )
                    nc.sync.dma_start(out=out[h, qh], in_=res_sb)

                return norm

            if cfg["reps"] == 0 and not cfg["mask_off"]:
                # ablation: mask chain only, no attention body
                load_mask()

            for rep in range(cfg["reps"]):
              for h in range(HPC):
                qT_sb = qk_pool.tile([D, S], qk_dt, tag="q", name=f"qT_{rep}_{h}")
                kT_sb = qk_pool.tile([D, S], qk_dt, tag="k", name=f"kT_{rep}_{h}")
                nc.sync.dma_start(out=qT_sb, in_=qT[h])
                nc.sync.dma_start(out=kT_sb, in_=kT[h])
                v_sb = v_pool.tile([128, KB, D + 1], v_dt, tag="v", name=f"v_{rep}_{h}")
                nc.sync.dma_start(out=v_sb, in_=v[h])
                if rep == 0 and h == 0 and not cfg["mask_off"]:
                    # mask DMAs go after h0's Q/K/V so the first head can
                    # start computing while mask blocks stream in.
                    # packed-u8 reads cut DRAM traffic 8x; unpack to bf16
                    # on-chip so the vector engines stay on their fast
                    # tensor-tensor path. (SWDGE cast-DMA = Q7 software
                    # cast, ~12ms.)
                    if cfg["mask_swdge"]:
                        for kb in range(KB):
                            nc.gpsimd.dma_start(
                                out=mask_sb[:, kb, :], in_=mT[:, kb, :]
                            )
                    else:
                        load_mask()
                    if mask_on_pe:
                        for kb in range(KB):
                            # {0,1} -> {-30000, 0}: additive mask; exp of a
                            # masked score underflows to exactly 0.0
                            nc.vector.tensor_scalar(
                                mask_sb[:, kb, :],
                                mask_sb[:, kb, :],
                                1.0,
                                30000.0,
                                mybir.AluOpType.subtract,
                                mybir.AluOpType.mult,
                            )

                for qh in range(QH):
                    q0 = qh * QHS
                    acc = ps_acc.tile(
                        [D + 1, QHS], F32, tag="acc", name=f"acc_{rep}_{h}_{qh}"
                    )

                    def make_pv(kb, pm_t, acc=acc, v_sb=v_sb):
                        def pv():
                            for j in range(2):
                                nc.tensor.matmul(
                                    acc[:, j * 512 : (j + 1) * 512],
                                    v_sb[:, kb, :],
                                    pm_t[:, j * 512 : (j + 1) * 512],
                                    start=(kb == 0),
                                    stop=(kb == KB - 1),
                                )

                        return pv

                    for kb in range(KB):
                        s_t = ps_s.tile(
                            [128, QHS], F32, tag="s", name=f"s_{rep}_{h}_{qh}_{kb}"
                        )
                        for j in range(2):
                            nc.tensor.matmul(
                                s_t[:, j * 512 : (j + 1) * 512],
                                kT_sb[:, kb * 128 : (kb + 1) * 128],
                                qT_sb[:, q0 + j * 512 : q0 + (j + 1) * 512],
                                start=True,
                                stop=not mask_on_pe,
                            )
                        if mask_on_pe:
                            for j in range(2):
                                nc.tensor.matmul(
                                    s_t[:, j * 512 : (j + 1) * 512],
                                    identity_bf,
                                    mask_sb[:, kb, q0 + j * 512 : q0 + (j + 1) * 512],
                                    start=False,
                                    stop=True,
                                )
                        drain_pe(skew)
                        p_t = p_pool.tile(
                            [128, QHS], p_dt, tag="p", name=f"p_{rep}_{h}_{qh}_{kb}"
                        )
                        nc.scalar.activation(
                            p_t, s_t, mybir.ActivationFunctionType.Exp, scale=0.125
                        )
                        if mask_on_pe:
                            pm_t = p_t
                        else:
                            m_ap = mask_sb[:, kb, q0 : q0 + QHS]
                            pm_t = pm_pool.tile(
                                [128, QHS], p_dt, tag="pm", name=f"pm_{rep}_{h}_{qh}_{kb}"
                            )
                            nc.vector.tensor_mul(
                                pm_t[:, 0:dve_cols], p_t[:, 0:dve_cols], m_ap[:, 0:dve_cols]
                            )
                            if dve_cols < QHS:
                                nc.gpsimd.tensor_mul(
                                    pm_t[:, dve_cols:], p_t[:, dve_cols:], m_ap[:, dve_cols:]
                                )
                        pe_queue.append(make_pv(kb, pm_t))
                    pe_queue.append(make_norm(h, qh, q0, acc, f"{rep}_{h}"))
            drain_pe(0)
    _NC_CACHE[key] = nc
    return nc


def make_in_maps(encodings_q, encodings_k, encodings_v, mask, **overrides):
    cfg = dict(CONFIG)
    cfg.update(overrides)
    v_np_dt = ml_dtypes.bfloat16 if cfg["p_bf16"] else np.float32
    qk_np_dt = ml_dtypes.bfloat16 if cfg["qk_bf16"] else np.float32
    in_maps = []
    maskT_by_b = {}
    for b in range(B):
        mT = np.ascontiguousarray(mask[b, 0].T)  # [k, q] bool
        if cfg["mask_bits"]:
            packed = np.packbits(mT, axis=1, bitorder="little")  # [S, S//8]
            maskT_by_b[b] = np.ascontiguousarray(
                packed.reshape(KB, 128, S // 8).transpose(1, 0, 2)
            )
        else:
            maskT_by_b[b] = mT.astype(np.uint8)
    for c in range(N_CORES):
        b = c // (N_CORES // B)
        h0 = (c % (N_CORES // B)) * HPC
        in_maps.append(
            {
                "qT": np.ascontiguousarray(
                    encodings_q[b, h0 : h0 + HPC].transpose(0, 2, 1)
                ).astype(qk_np_dt),
                "kT": np.ascontiguousarray(
                    encodings_k[b, h0 : h0 + HPC].transpose(0, 2, 1)
                ).astype(qk_np_dt),
                "v": np.ascontiguousarray(
                    np.concatenate(
                        [
                            encodings_v[b, h0 : h0 + HPC],
                            np.ones((HPC, S, 1), np.float32),
                        ],
                        axis=-1,
                    )
                    .reshape(HPC, KB, 128, D + 1)
                    .transpose(0, 2, 1, 3)
                ).astype(v_np_dt),
                "maskT": maskT_by_b[b],
            }
        )
    return in_maps


def gather_out(results):
    out = np.empty((B, H, S, D), np.float32)
    for c in range(N_CORES):
        b = c // (N_CORES // B)
        h0 = (c % (N_CORES // B)) * HPC
        # out dram is [h, qh, p, j, d] with q = qh*QHS + j*128 + p
        arr = np.asarray(results[c]["out"], dtype=np.float32)
        out[b, h0 : h0 + HPC] = arr.transpose(0, 1, 3, 2, 4).reshape(HPC, S, D)
    return out


def kernel(encodings_q, encodings_k, encodings_v, mask):
    encodings_q = np.asarray(encodings_q, dtype=np.float32)
    encodings_k = np.asarray(encodings_k, dtype=np.float32)
    encodings_v = np.asarray(encodings_v, dtype=np.float32)
    mask = np.asarray(mask)
    nc = build_nc()
    in_maps = make_in_maps(encodings_q, encodings_k, encodings_v, mask)
    res = run_bass_kernel_spmd(nc, in_maps, core_ids=list(range(N_CORES)))
    return gather_out(res.results)



# revision 32
# speedup vs baseline: 1.0094x; 1.0094x over previous
"""Trainium2 Bass kernel: batched multi-head scaled-dot-product attention.

Problem shapes: Q/K/V [4, 16, 2048, 64] f32, mask [4, 1, 2048, 2048] bool.
out = softmax(Q K^T / 8 + mask) V.

Sharding: 8 cores; core c handles batch b = c//2, heads (c%2)*8 .. +8.
Each (b, h) is independent; the mask is shared across the 8 heads on a core.

Per-core kernel (per head h, per query-half qh of 1024):
  S^T[k, q]  = K Q^T         (PE; d=64 on partitions, 16 key-blocks of 128)
  P^T        = exp(S^T / 8)  (ACT, from PSUM)
  PM^T       = P^T * mask^T  (0/1 multiply on DVE; exact)
  acc[d', q] = sum_k V'[k, d'] PM^T[k, q]  (PE, PSUM accumulate over k-blocks;
               V' has a ones column so acc[64, q] = softmax denominator)
  out[q, d]  = transpose(acc) * 1/acc[64]  (PE transpose + DVE recip/mul)

HW-measured design notes (axon trn2, 2026-08; sim alone is misleading):
  - All matmul operands bf16 (q/k/v ship as bf16): f32r matmuls and f32
    DVE tensor ops are several-x slower on real HW than the cost model
    says; bf16 cut the measured body ~2.08ms -> ~0.33ms (rel err 5e-3,
    tolerance 2e-2). Body is now ~10% above the ACT exp-busy floor.
  - Never DMA-cast via nc.gpsimd (SWDGE): the Q7 does the cast in
    software, ~12ms for the 8MB mask. Mask ships bit-packed (512KB/core),
    HWDGE-loaded, bit-extracted + cast to bf16 {0,1} on DVE (exact).
  - All DRAM<->SBUF transfers are host-pre-permuted so every partition
    reads/writes one contiguous >=2KB run (128 descriptors per DMA, not
    1-2k small strided ones); out is stored [h, qh, p, j, d] bf16 and
    unpermuted/cast on host.
"""

import numpy as np
import ml_dtypes

import bass_rust
import concourse.bass as bass
import concourse.mybir as mybir
import concourse.tile as tile
from concourse.bass_utils import run_bass_kernel_spmd
from concourse.masks import make_identity

B, H, S, D = 4, 16, 2048, 64
N_CORES = 8
HPC = H // (N_CORES // B)  # heads per core = 8
KB = S // 128  # 16 key blocks
QH = 2  # query halves
QHS = S // QH  # 1024
F32 = mybir.dt.float32
F32R = mybir.dt.float32r
BF16 = mybir.dt.bfloat16

# Kernel configuration (tuned via TimelineSim + hardware runs).
CONFIG = {
    "p_bf16": True,  # P (post-exp) and V in bf16: faster DVE mask, ~10x err
    "dve_cols": 1024,  # mask columns (of 1024) on DVE; GPSIMD op was slow on HW
    "p_bufs": 10,
    "s_bufs": 2,
    "pv_skew": 2,  # deferred-PE-queue depth (PE is in-order)
    "reps": 1,  # repeat the whole body (timing experiments only)
    "norm_on_act": False,  # ScalarE copies measured neutral; keep DVE path
    "mask_on_pe": False,  # PE-side additive mask measured +51us/body on HW
    "mask_swdge": False,  # SWDGE cast-DMA: Q7 does the u8->bf16 cast in
    # software, ~12ms one-time on HW (sim doesn't model it). False = HWDGE
    # u8 load + on-chip DVE cast.
    "qk_bf16": True,  # Q/K in bf16: PE QK^T at true 1 cyc/row on HW
    "mask_off": False,  # timing ablation: skip mask load + cast chain
    "mask_bits": True,  # ship mask bit-packed (S/8 bytes/row), unpack on DVE
    "out_bf16": True,  # store out bf16: halves D2H, ~2e-3 extra rounding
}


def _patched_drain_and_barrier(self, tick_clock, wait_clock):
    """This neuronxcc's CoreV3 codegen allows only 1 sync-wait per TPB_CTRL
    instruction; Tile's end-of-kernel drain can carry many. Split them."""
    drain_inst = self.nc.sync.drain()
    wait_clock.add_sem_waits(
        drain_inst.ins, tile.ScopedClock({None: tick_clock.global_clock})
    )
    mi = drain_inst.ins
    si = mi.sync_info
    waits = list(si.on_wait) if si is not None else []
    if len(waits) > 1:
        si.on_wait = waits[:1]
        mi.sync_info = si
        for i in range(1, len(waits)):
            extra = self.nc.sync.drain()
            extra.ins.sync_info = bass_rust.SyncInfo(
                on_wait=waits[i : i + 1], on_update=[]
            )
    self.nc.all_engine_barrier()
    popped = self.nc._tile_sem_poison_stack.pop()
    assert popped is self._sem_poison
    self.nc.clear_and_free_semaphores(list(self.sems.allocated().values()))
    self.nc.all_engine_barrier()


tile.TileContext._drain_and_barrier = _patched_drain_and_barrier

_ORIG_COMMIT = tile.TileContext._commit_instruction


def _commit_split_waits(self, inst, lazy_reg_writes=True):
    """Hoist all-but-one sem wait of an instruction onto single-wait NoOp
    carriers on the same engine (same 1-wait codegen limit as above).
    DMACopy lowers to SP pseudo-DMA instructions dispatched in program order
    by the SP sequencer, so carriers gate those too."""
    si = getattr(inst, "sync_info", None)
    if (
        si is not None
        and len(si.on_wait) > 1
        and inst.engine != mybir.EngineType.Unassigned
    ):
        waits = list(si.on_wait)
        for w in waits[:-1]:
            nop = mybir.InstNoOp(name=self.nc.get_next_instruction_name())
            nop.engine = inst.engine
            nop.sync_info = bass_rust.SyncInfo(on_wait=[w], on_update=[])
            self._add_instruction(nop)
        si.on_wait = waits[-1:]
        inst.sync_info = si
    return _ORIG_COMMIT(self, inst, lazy_reg_writes)


tile.TileContext._commit_instruction = _commit_split_waits

_NC_CACHE = {}


def build_nc(**overrides):
    cfg = dict(CONFIG)
    cfg.update(overrides)
    key = tuple(sorted(cfg.items()))
    if key in _NC_CACHE:
        return _NC_CACHE[key]
    p_dt = BF16 if cfg["p_bf16"] else F32R
    v_dt = BF16 if cfg["p_bf16"] else F32R
    qk_dt = BF16 if cfg["qk_bf16"] else F32R
    dve_cols = cfg["dve_cols"]

    nc = bass.Bass("TRN2", target_bir_lowering=False, debug=False, num_devices=N_CORES)
    qT = nc.dram_tensor("qT", [HPC, D, S], qk_dt, kind="ExternalInput")
    kT = nc.dram_tensor("kT", [HPC, D, S], qk_dt, kind="ExternalInput")
    # v pre-permuted on host to [h, p, kb, d] (p = key % 128, kb = key // 128)
    # so the SBUF load is 128 contiguous per-partition descriptors, not 2048
    # tiny strided ones. out likewise stored as [h, qh, p, j, d] (q = j*128+p)
    # and unpermuted on host.
    v = nc.dram_tensor("v", [HPC, 128, KB, D + 1], v_dt, kind="ExternalInput")
    if cfg["mask_bits"]:
        # bit-packed + host-permuted: maskT[p, kb, i] byte holds keys
        # kb*128+p, queries i*8..i*8+7 (little-endian bit order)
        maskT = nc.dram_tensor(
            "maskT", [128, KB, S // 8], mybir.dt.uint8, kind="ExternalInput"
        )
    else:
        maskT = nc.dram_tensor("maskT", [S, S], mybir.dt.uint8, kind="ExternalInput")
    out_dt = BF16 if cfg["out_bf16"] else F32
    out = nc.dram_tensor(
        "out", [HPC, QH, 128, QHS // 128, D], out_dt, kind="ExternalOutput"
    )

    with tile.TileContext(nc) as tc:
        with (
            tc.tile_pool(name="consts", bufs=1) as consts,
            tc.tile_pool(name="qk", bufs=2) as qk_pool,
            tc.tile_pool(name="vp", bufs=2) as v_pool,
            tc.tile_pool(name="pp", bufs=cfg["p_bufs"]) as p_pool,
            tc.tile_pool(name="pm", bufs=cfg["p_bufs"]) as pm_pool,
            tc.tile_pool(name="op", bufs=2) as o_pool,
            tc.tile_pool(name="small", bufs=2) as small,
            tc.tile_pool(name="res", bufs=2) as res_pool,
            tc.tile_pool(name="ps_s", bufs=cfg["s_bufs"], space="PSUM") as ps_s,
            tc.tile_pool(name="ps_acc", bufs=1, space="PSUM") as ps_acc,
            tc.tile_pool(name="ps_tr", bufs=1, space="PSUM") as ps_tr,
        ):
            identity = consts.tile([128, 128], F32)
            make_identity(nc, identity)
            mask_on_pe = cfg["mask_on_pe"]
            if mask_on_pe:
                identity_bf = consts.tile([128, 128], BF16)
                make_identity(nc, identity_bf)
            mask_sb = consts.tile([128, KB, S], BF16)
            if cfg["mask_bits"]:
                mT = maskT
            else:
                mT = maskT.rearrange("(n p) q -> p n q", p=128)

            def load_mask():
                if cfg["mask_bits"]:
                    mu8p = consts.tile(
                        [128, KB, S // 8], mybir.dt.uint8, name="mask_u8p"
                    )
                    nc.sync.dma_start(out=mu8p, in_=mT[:, :, :])
                    mu8 = consts.tile([128, KB, S], mybir.dt.uint8, name="mask_u8")
                    # chunk by key-block group so kb 0-3's mask is ready
                    # early and the first head's PM muls aren't gated on the
                    # whole unpack
                    for c4 in range(4):
                        kbs = slice(c4 * 4, (c4 + 1) * 4)
                        src = mu8p[:, kbs, :].rearrange("p n i -> p (n i)")
                        dst = mu8[:, kbs, :].rearrange("p n (i j) -> p (n i) j", j=8)
                        for j in range(8):
                            # bitVec ops can't cast, so extract bits u8->u8
                            nc.vector.tensor_scalar(
                                dst[:, :, j],
                                src,
                                j,
                                1,
                                mybir.AluOpType.logical_shift_right,
                                mybir.AluOpType.bitwise_and,
                            )
                        nc.vector.tensor_copy(mask_sb[:, kbs, :], mu8[:, kbs, :])
                else:
                    mu8 = consts.tile([128, KB, S], mybir.dt.uint8, name="mask_u8")
                    for c4 in range(4):
                        kbs = slice(c4 * 4, (c4 + 1) * 4)
                        nc.sync.dma_start(out=mu8[:, kbs, :], in_=mT[:, kbs, :])
                        nc.vector.tensor_copy(mask_sb[:, kbs, :], mu8[:, kbs, :])

            skew = cfg["pv_skew"]
            # Deferred PE work (PV accumulates + normalizes), drained one item
            # per QK slot including across (h, qh) boundaries — keeps the
            # in-order PE from stalling on a just-produced dependency.
            from collections import deque

            pe_queue = deque()

            def drain_pe(target):
                while len(pe_queue) > target:
                    pe_queue.popleft()()

            def make_norm(h, qh, q0, acc, ntag):
                def norm():
                    # normalize: transpose acc to [q, d], divide by ones-col
                    o_sb = o_pool.tile([D + 1, QHS], F32, tag="o", name=f"o_{ntag}_{qh}")
                    cp = nc.scalar if cfg["norm_on_act"] else nc.vector
                    cp.copy(o_sb, acc) if cfg["norm_on_act"] else cp.tensor_copy(o_sb, acc)
                    tr = ps_tr.tile([128, 8, 128], F32, tag="tr", name=f"tr_{ntag}_{qh}")
                    for j in range(8):
                        nc.tensor.transpose(
                            tr[:, j, 0 : D + 1],
                            o_sb[:, j * 128 : (j + 1) * 128],
                            identity[0 : D + 1, 0 : D + 1],
                        )
                    den_sb = small.tile([128, 8], F32, tag="den", name=f"den_{ntag}_{qh}")
                    if cfg["norm_on_act"]:
                        nc.scalar.copy(den_sb, tr[:, :, D])
                    else:
                        nc.vector.tensor_copy(den_sb, tr[:, :, D])
                    rec_sb = small.tile([128, 8], F32, tag="rec", name=f"rec_{ntag}_{qh}")
                    nc.vector.reciprocal(rec_sb, den_sb)
                    res_sb = res_pool.tile(
                        [128, 8, D], out_dt, tag="res", name=f"res_{ntag}_{qh}"
                    )
                    for j in range(8):
                        nc.vector.tensor_scalar_mul(
                            res_sb[:, j, :], tr[:, j, 0:D], rec_sb[:, j : j + 1]
                        )
                    nc.sync.dma_start(out=out[h, qh], in_=res_sb)

                return norm

            if cfg["reps"] == 0 and not cfg["mask_off"]:
                # ablation: mask chain only, no attention body
                load_mask()

            for rep in range(cfg["reps"]):
              for h in range(HPC):
                qT_sb = qk_pool.tile([D, S], qk_dt, tag="q", name=f"qT_{rep}_{h}")
                kT_sb = qk_pool.tile([D, S], qk_dt, tag="k", name=f"kT_{rep}_{h}")
                nc.sync.dma_start(out=qT_sb, in_=qT[h])
                nc.sync.dma_start(out=kT_sb, in_=kT[h])
                v_sb = v_pool.tile([128, KB, D + 1], v_dt, tag="v", name=f"v_{rep}_{h}")
                nc.sync.dma_start(out=v_sb, in_=v[h])
                if rep == 0 and h == 0 and not cfg["mask_off"]:
                    # mask DMAs go after h0's Q/K/V so the first head can
                    # start computing while mask blocks stream in.
                    # packed-u8 reads cut DRAM traffic 8x; unpack to bf16
                    # on-chip so the vector engines stay on their fast
                    # tensor-tensor path. (SWDGE cast-DMA = Q7 software
                    # cast, ~12ms.)
                    if cfg["mask_swdge"]:
                        for kb in range(KB):
                            nc.gpsimd.dma_start(
                                out=mask_sb[:, kb, :], in_=mT[:, kb, :]
                            )
                    else:
                        load_mask()
                    if mask_on_pe:
                        for kb in range(KB):
                            # {0,1} -> {-30000, 0}: additive mask; exp of a
                            # masked score underflows to exactly 0.0
                            nc.vector.tensor_scalar(
                                mask_sb[:, kb, :],
                                mask_sb[:, kb, :],
                                1.0,
                                30000.0,
                                mybir.AluOpType.subtract,
                                mybir.AluOpType.mult,
                            )

                for qh in range(QH):
                    q0 = qh * QHS
                    acc = ps_acc.tile(
                        [D + 1, QHS], F32, tag="acc", name=f"acc_{rep}_{h}_{qh}"
                    )

                    def make_pv(kb, pm_t, acc=acc, v_sb=v_sb):
                        def pv():
                            for j in range(2):
                                nc.tensor.matmul(
                                    acc[:, j * 512 : (j + 1) * 512],
                                    v_sb[:, kb, :],
                                    pm_t[:, j * 512 : (j + 1) * 512],
                                    start=(kb == 0),
                                    stop=(kb == KB - 1),
                                )

                        return pv

                    for kb in range(KB):
                        s_t = ps_s.tile(
                            [128, QHS], F32, tag="s", name=f"s_{rep}_{h}_{qh}_{kb}"
                        )
                        for j in range(2):
                            nc.tensor.matmul(
                                s_t[:, j * 512 : (j + 1) * 512],
                                kT_sb[:, kb * 128 : (kb + 1) * 128],
                                qT_sb[:, q0 + j * 512 : q0 + (j + 1) * 512],
                                start=True,
                                stop=not mask_on_pe,
                            )
                        if mask_on_pe:
                            for j in range(2):
                                nc.tensor.matmul(
                                    s_t[:, j * 512 : (j + 1) * 512],
                                    identity_bf,
                                    mask_sb[:, kb, q0 + j * 512 : q0 + (j + 1) * 512],
                                    start=False,
                                    stop=True,
                                )
                        drain_pe(skew)
                        p_t = p_pool.tile(
                            [128, QHS], p_dt, tag="p", name=f"p_{rep}_{h}_{qh}_{kb}"
                        )
                        nc.scalar.activation(
                            p_t, s_t, mybir.ActivationFunctionType.Exp, scale=0.125
                        )
                        if mask_on_pe:
                            pm_t = p_t
                        else:
                            m_ap = mask_sb[:, kb, q0 : q0 + QHS]
                            pm_t = pm_pool.tile(
                                [128, QHS], p_dt, tag="pm", name=f"pm_{rep}_{h}_{qh}_{kb}"
                            )
                            nc.vector.tensor_mul(
                                pm_t[:, 0:dve_cols], p_t[:, 0:dve_cols], m_ap[:, 0:dve_cols]
                            )
                            if dve_cols < QHS:
                                nc.gpsimd.tensor_mul(
                                    pm_t[:, dve_cols:], p_t[:, dve_cols:], m_ap[:, dve_cols:]
                                )
                        pe_queue.append(make_pv(kb, pm_t))
                    pe_queue.append(make_norm(h, qh, q0, acc, f"{rep}_{h}"))
            drain_pe(0)
    _NC_CACHE[key] = nc
    return nc


def make_in_maps(encodings_q, encodings_k, encodings_v, mask, **overrides):
    cfg = dict(CONFIG)
    cfg.update(overrides)
    v_np_dt = ml_dtypes.bfloat16 if cfg["p_bf16"] else np.float32
    qk_np_dt = ml_dtypes.bfloat16 if cfg["qk_bf16"] else np.float32
    in_maps = []
    maskT_by_b = {}
    for b in range(B):
        mT = np.ascontiguousarray(mask[b, 0].T)  # [k, q] bool
        if cfg["mask_bits"]:
            packed = np.packbits(mT, axis=1, bitorder="little")  # [S, S//8]
            maskT_by_b[b] = np.ascontiguousarray(
                packed.reshape(KB, 128, S // 8).transpose(1, 0, 2)
            )
        else:
            maskT_by_b[b] = mT.astype(np.uint8)
    for c in range(N_CORES):
        b = c // (N_CORES // B)
        h0 = (c % (N_CORES // B)) * HPC
        in_maps.append(
            {
                "qT": np.ascontiguousarray(
                    encodings_q[b, h0 : h0 + HPC].transpose(0, 2, 1)
                ).astype(qk_np_dt),
                "kT": np.ascontiguousarray(
                    encodings_k[b, h0 : h0 + HPC].transpose(0, 2, 1)
                ).astype(qk_np_dt),
                "v": np.ascontiguousarray(
                    np.concatenate(
                        [
                            encodings_v[b, h0 : h0 + HPC],
                            np.ones((HPC, S, 1), np.float32),
                        ],
                        axis=-1,
                    )
                    .reshape(HPC, KB, 128, D + 1)
                    .transpose(0, 2, 1, 3)
                ).astype(v_np_dt),
                "maskT": maskT_by_b[b],
            }
        )
    return in_maps


def gather_out(results):
    out = np.empty((B, H, S, D), np.float32)
    for c in range(N_CORES):
        b = c // (N_CORES // B)
        h0 = (c % (N_CORES // B)) * HPC
        # out dram is [h, qh, p, j, d] with q = qh*QHS + j*128 + p
        arr = np.asarray(results[c]["out"], dtype=np.float32)
        out[b, h0 : h0 + HPC] = arr.transpose(0, 1, 3, 2, 4).reshape(HPC, S, D)
    return out


def kernel(encodings_q, encodings_k, encodings_v, mask):
    encodings_q = np.asarray(encodings_q, dtype=np.float32)
    encodings_k = np.asarray(encodings_k, dtype=np.float32)
    encodings_v = np.asarray(encodings_v, dtype=np.float32)
    mask = np.asarray(mask)
    nc = build_nc()
    in_maps = make_in_maps(encodings_q, encodings_k, encodings_v, mask)
    res = run_bass_kernel_spmd(nc, in_maps, core_ids=list(range(N_CORES)))
    return gather_out(res.results)

